# revision 18
# baseline (speedup 1.0000x reference)
"""Trainium2 Bass kernel for nn_Attention_91293824844283.

Multi-head attention (identity rep): per-head 1x1-conv Q/K/V projections,
softmax(Q K^T / sqrt(E)) V, per-head output projection summed over heads.

Shapes: B=4, N=2048, D=512, H=8, E=64.

Sharding over 8 cores: core c -> (batch b = c//2, head-group g = c%2 of 4
heads). Each core computes the partial output sum over its 4 heads for its
batch; host adds the two partials per batch.

Device-side design (per core), v2:
  - Layouts as before: x^T inputs (bf16), packed transposed weights,
    1/sqrt(E) folded into Wq. V augmented with a ones column (slot 66) so
    the PV matmul also emits softmax denominators (M=65).
  - Scalar-engine exp is the pacer (128 ACTIVATEs x ~1.1us). All other
    work is emitted INSIDE the exp-paced sweeps so the in-order engine
    queues never stall ACT: quarter q's normalization + output projection
    run as PE/DVE fillers interleaved into quarter q+1's sweep.
  - Transpose-free normalization: rep stays unnormalized; r = 1/sums row
    (DVE reciprocal on [1,512]), PE outer-product broadcasts r to [64,512],
    one DVE multiply writes pre-normalized bf16 rep^T into packed pair
    tiles (head s at partitions s*64).
  - Output projection per 128-query tile: two concurrent K=64 row-tiled
    matmul chains (heads 0/2 -> bank A via rows 0:64, heads 1/3 -> bank B
    via rows 64:128), ost = A + B on DVE, DMA out.
  - Ramp: chunked K/Q/V projections interleaved with the first sweep's
    tiles so the first exp fires ~5us in.
"""

import numpy as np
import ml_dtypes
from contextlib import ExitStack

B, N, D, H, E = 4, 2048, 512, 8, 64
HPC = 4            # heads per core
N_CORES = 8
NKT = N // 128     # 16 nk tiles
VSLOT = 66         # V slot: 64 V cols + 1 ones col + 1 pad
KT = D // 128      # 4 contraction tiles for projections
QW = 512           # nq quarter width

_CACHE = {}


def _build():
    import concourse.tile as tile
    from concourse import bacc, mybir

    bf16 = mybir.dt.bfloat16
    f32 = mybir.dt.float32
    Exp = mybir.ActivationFunctionType.Exp

    nc = bacc.Bacc(
        "TRN2", target_bir_lowering=False, debug=False, num_devices=N_CORES
    )
    # chunk-major x layouts: [chunk c, 128, KT*512] so each chunk is ONE
    # contiguous 512KB DMA (serial dma_starts on the issuing engine block
    # for ~transfer time — fewer, bigger transfers keep the queue short).
    xqT = nc.dram_tensor("xqT", [4, 128, KT * 512], bf16, kind="ExternalInput").ap()
    xkT = nc.dram_tensor("xkT", [4, 128, KT * 512], bf16, kind="ExternalInput").ap()
    vT = nc.dram_tensor("vT", [4, 128, KT * 512], bf16, kind="ExternalInput").ap()
    wqT = nc.dram_tensor("wqT", [2, 128, KT * 128], bf16, kind="ExternalInput").ap()
    wkT = nc.dram_tensor("wkT", [2, 128, KT * 128], bf16, kind="ExternalInput").ap()
    wvT = nc.dram_tensor("wvT", [128, KT * HPC * E], bf16, kind="ExternalInput").ap()
    woP = nc.dram_tensor("woP", [2, 128, D], bf16, kind="ExternalInput").ap()
    outp = nc.dram_tensor("outp", [NKT, 128, D], f32, kind="ExternalOutput").ap()

    with tile.TileContext(nc) as tc, ExitStack() as ctx:
        cp = ctx.enter_context(tc.tile_pool(name="const", bufs=1))

        # --- persistent SBUF tiles (chunk-major x: tile c holds all KT
        # k-subtiles of 512 columns each) ---
        xq = [cp.tile([128, KT * 512], bf16, tag=f"xq{c}", name=f"xq{c}")
              for c in range(4)]
        xk = [cp.tile([128, KT * 512], bf16, tag=f"xk{c}", name=f"xk{c}")
              for c in range(4)]
        xv = [cp.tile([128, KT * 512], bf16, tag=f"xv{c}", name=f"xv{c}")
              for c in range(4)]
        wq = [cp.tile([128, KT * 128], bf16, tag=f"wq{p}", name=f"wq{p}")
              for p in range(2)]
        wk = [cp.tile([128, KT * 128], bf16, tag=f"wk{p}", name=f"wk{p}")
              for p in range(2)]
        wv = cp.tile([128, KT * HPC * E], bf16, tag="wv", name="wv")
        wo = [cp.tile([128, D], bf16, tag=f"wo{p}", name=f"wo{p}") for p in range(2)]
        qt = [cp.tile([128, N], bf16, tag=f"qt{p}", name=f"qt{p}") for p in range(2)]
        kt = [cp.tile([128, N], bf16, tag=f"kt{p}", name=f"kt{p}") for p in range(2)]
        vaug = [cp.tile([128, HPC * VSLOT], bf16, tag=f"va{t}", name=f"va{t}")
                for t in range(NKT)]
        # pre-normalized rep^T, packed pairs: head 2p+s at partitions s*64.
        # s=0 is written directly by DVE; s=1 lands in rsh (partitions
        # 0:64) and is shifted up via SBUF->SBUF DMA (DVE lanes are
        # partition-aligned; DMA can cross partitions).
        repP = [cp.tile([128, N], bf16, tag=f"rp{p}", name=f"repP{p}")
                for p in range(2)]
        rsh = [[cp.tile([E, QW], bf16, tag=f"rsh{par}{s}", name=f"rsh{par}{s}")
                for s in range(2)] for par in range(2)]
        # drain staging (double buffered by quarter parity)
        rts = [[cp.tile([65, QW], f32, tag=f"rts{par}{s}", name=f"rts{par}{s}")
                for s in range(2)] for par in range(2)]
        rcb = [[cp.tile([1, QW], bf16, tag=f"rcb{par}{s}", name=f"rcb{par}{s}")
                for s in range(2)] for par in range(2)]
        # [1,512] DVE reciprocal is pathological (one lane, iterative op,
        # ~3.3us). Round-trip the sums row through a [128,4] layout via
        # SBUF->SBUF DMAs (scatter p-major, gather back the same way — the
        # intermediate layout cancels) so recip runs on 128 lanes (~90ns).
        dT = [[cp.tile([128, 4], f32, tag=f"dT{par}{s}", name=f"dT{par}{s}")
               for s in range(2)] for par in range(2)]
        rT = [[cp.tile([128, 4], f32, tag=f"rT{par}{s}", name=f"rT{par}{s}")
               for s in range(2)] for par in range(2)]
        rTb = [[cp.tile([128, 4], bf16, tag=f"rTb{par}{s}", name=f"rTb{par}{s}")
                for s in range(2)] for par in range(2)]
        ones64 = cp.tile([1, 64], bf16, tag="ones64")
        dmy = cp.tile([128, 1], f32, tag="dmy")
        dmyo = cp.tile([128, 1], bf16, tag="dmyo")

        # --- exp table preload: first ACTIVATE triggers ACT_TABLE_LOAD
        # (~2.7us); fire it immediately so it overlaps the input DMA.
        nc.gpsimd.memset(dmy[:], 0.0)
        nc.scalar.activation(dmyo[:], dmy[:], Exp)
        nc.gpsimd.memset(ones64[:], 1.0)
        for t in range(NKT):
            nc.gpsimd.memset(vaug[t][:], 1.0)

        # --- input DMAs, ordered to unblock the pipelined ramp:
        # pair-0 K weights + xk c0 first, then Q/V weights + c0, then the
        # remaining K/V chunks (vproj tiles follow the sweep), Q c1-3,
        # pair-1 weights, output-proj weights.
        # ALL input DMAs go on the sync queue: a dma_start occupies the
        # issuing engine for roughly the transfer time, so putting any on
        # the scalar queue would push the first exp out by ~20us.
        nc.sync.dma_start(wk[0][:], wkT[0])
        nc.sync.dma_start(xk[0][:], xkT[0])
        nc.sync.dma_start(wq[0][:], wqT[0])
        nc.sync.dma_start(xq[0][:], xqT[0])
        nc.sync.dma_start(wv[:], wvT[:])
        nc.sync.dma_start(xv[0][:], vT[0])
        for c in range(1, 4):
            nc.sync.dma_start(xk[c][:], xkT[c])
            nc.sync.dma_start(xv[c][:], vT[c])
        for c in range(1, 4):
            nc.sync.dma_start(xq[c][:], xqT[c])
        nc.sync.dma_start(wk[1][:], wkT[1])
        nc.sync.dma_start(wq[1][:], wqT[1])
        for p in range(2):
            nc.sync.dma_start(wo[p][:], woP[p])

        # --- small PE warmup burst during the initial DMA window.
        warm_sb = cp.tile([128, 512], bf16, tag="warm_sb")
        nc.gpsimd.memset(warm_sb[:], 0.0)
        with tc.tile_pool(name="warmps", bufs=1, space="PSUM") as wps:
            wpt = wps.tile([128, 512], f32, tag="w", name="warm_ps")
            for i in range(12):
                nc.tensor.matmul(wpt[:], warm_sb[:, 0:128], warm_sb[:],
                                 start=True, stop=True)

        # --- pools. PSUM: s pair tile 2 banks x bufs=2 + rep 2x1 bank +
        # fill 2x1 = 8 banks.
        sp = ctx.enter_context(tc.tile_pool(name="spsum", bufs=2, space="PSUM"))
        rp = ctx.enter_context(tc.tile_pool(name="rpsum", bufs=1, space="PSUM"))
        fpp = ctx.enter_context(tc.tile_pool(name="fill", bufs=2, space="PSUM"))
        ptp = ctx.enter_context(tc.tile_pool(name="ptile", bufs=4))
        ostp = ctx.enter_context(tc.tile_pool(name="ostp", bufs=2))

        def proj_chunk(dst, w, x, c):
            ps = fpp.tile([128, 512], f32, tag="f", name="proj_ps")
            for k in range(KT):
                nc.tensor.matmul(
                    ps[:], w[:, k * 128:(k + 1) * 128],
                    x[c][:, k * 512:(k + 1) * 512],
                    start=(k == 0), stop=(k == KT - 1),
                )
            nc.vector.tensor_copy(dst[:, c * 512:(c + 1) * 512], ps[:])

        def vproj(t):
            ps = fpp.tile([128, HPC * E], f32, tag="f", name="vproj_ps")
            c, off = t // 4, (t % 4) * 128
            for k in range(KT):
                nc.tensor.matmul(
                    ps[:], xv[c][:, k * 512 + off:k * 512 + off + 128],
                    wv[:, k * HPC * E:(k + 1) * HPC * E],
                    start=(k == 0), stop=(k == KT - 1),
                )
            for h in range(HPC):
                nc.vector.tensor_copy(
                    vaug[t][:, h * VSLOT:h * VSLOT + E],
                    ps[:, h * E:(h + 1) * E],
                )

        def norm_part(p, q, s):
            # quarter (p,q) head s: rts holds unnormalized rep^T [65, 512]
            # (row 64 = softmax denominators). Write pre-normalized bf16
            # rep^T into repP[p] partitions s*64:(s+1)*64.
            par = (4 * p + q) % 2
            qsl = slice(q * QW, (q + 1) * QW)
            nc.vector.reciprocal(rT[par][s][:], dT[par][s][:])
            nc.vector.tensor_copy(rTb[par][s][:], rT[par][s][:])
            nc.sync.dma_start(rcb[par][s][:], rTb[par][s][:])
            bc = fpp.tile([64, QW], f32, tag="f", name="bcast")
            nc.tensor.matmul(bc[:], ones64[:], rcb[par][s][:],
                             start=True, stop=True)
            if s == 0:
                nc.vector.tensor_mul(repP[p][0:E, qsl],
                                     rts[par][s][0:E, :], bc[:])
            else:
                nc.vector.tensor_mul(rsh[par][1][:],
                                     rts[par][s][0:E, :], bc[:])
                nc.sync.dma_start(repP[p][E:128, qsl], rsh[par][1][:])

        def outproj(t):
            tsl = slice(t * 128, (t + 1) * 128)
            pa = fpp.tile([128, D], f32, tag="f", name="opA")
            pb = fpp.tile([128, D], f32, tag="f", name="opB")
            for p in range(2):
                nc.tensor.matmul(pa[:], repP[p][0:E, tsl], wo[p][0:E, :],
                                 start=(p == 0), stop=(p == 1))
                nc.tensor.matmul(pb[:], repP[p][E:128, tsl], wo[p][E:128, :],
                                 start=(p == 0), stop=(p == 1))
            osa = ostp.tile([128, D], f32, tag="osa", name="osa")
            nc.vector.tensor_copy(osa[:], pa[:])
            ost = ostp.tile([128, D], f32, tag="ost", name="ost")
            nc.vector.tensor_add(ost[:], osa[:], pb[:])
            nc.sync.dma_start(outp[t], ost[:])

        def sweep(p, q, pre=None, fillers=()):
            """One attention quarter: 16 x (S pair, exp, PV pair).

            pre: dict t -> list of callables emitted before tile t's S.
            fillers: list of (t, fn): fn is emitted after tile t's PV (it
            executes in engine gaps while ACT paces the sweep). Slots must
            be late enough that any DMA the fn depends on has landed —
            a premature emission stalls the whole in-order PE queue.
            """
            fq = {}
            for slot, fn in fillers:
                fq.setdefault(slot, []).append(fn)
            qoff = q * QW
            rep = [rp.tile([65, QW], f32, tag=f"rep{s}", name=f"rep{s}")
                   for s in range(2)]
            for t in range(NKT):
                if pre:
                    for fn in pre.get(t, ()):
                        fn()
                tsl = slice(t * 128, (t + 1) * 128)
                spair = sp.tile([128, 2 * QW], f32, tag="s", name="spair")
                for s in range(2):
                    esl = slice(s * 64, (s + 1) * 64)
                    nc.tensor.matmul(
                        spair[:, s * QW:(s + 1) * QW],
                        kt[p][esl, tsl], qt[p][esl, qoff:qoff + QW],
                        start=True, stop=True,
                    )
                pt = ptp.tile([128, 2 * QW], bf16, tag="p", name="pt")
                nc.scalar.activation(pt[:], spair[:], Exp)
                for s in range(2):
                    h = 2 * p + s
                    vsl = slice(h * VSLOT, h * VSLOT + 65)
                    nc.tensor.matmul(
                        rep[s][:],
                        vaug[t][:, vsl], pt[:, s * QW:(s + 1) * QW],
                        start=(t == 0), stop=(t == NKT - 1),
                    )
                for fn in fq.get(t, ()):
                    fn()
            # drain rep -> rts staging so the next quarter can reuse the
            # rep PSUM banks; also kick off the sums-row scatter so the
            # reciprocal chain is ready early next sweep. Consumers run as
            # fillers next sweep.
            par = (4 * p + q) % 2
            for s in range(2):
                nc.vector.tensor_copy(rts[par][s][:], rep[s][:])
                nc.sync.dma_start(dT[par][s][:], rts[par][s][64:65, :])

        # --- ramp: minimal work to start sweep (0,0) ---
        proj_chunk(kt[0], wk[0], xk, 0)
        proj_chunk(qt[0], wq[0], xq, 0)
        for t in range(4):
            vproj(t)

        # --- emission schedule ---
        # sweep (0,0): in-sweep K-chunk projections + vproj per tile
        pre00 = {}
        for c in range(1, 4):
            pre00.setdefault(4 * c, []).append(
                (lambda cc: lambda: proj_chunk(kt[0], wk[0], xk, cc))(c))
        for t in range(4, NKT):
            pre00.setdefault(t, []).append((lambda tt: lambda: vproj(tt))(t))
        # qproj c1's input DMA lands ~15us in — slot it late in the sweep
        # so the in-order PE queue never blocks on it.
        sweep(0, 0, pre=pre00,
              fillers=[(13, lambda: proj_chunk(qt[0], wq[0], xq, 1))])

        def nf(p, q, s):
            return lambda: norm_part(p, q, s)

        def pf(dst, w, x, c):
            return lambda: proj_chunk(dst, w, x, c)

        def of(t):
            return lambda: outproj(t)

        sweep(0, 1, fillers=[
            (3, nf(0, 0, 0)), (5, nf(0, 0, 1)),
            (7, pf(qt[0], wq[0], xq, 2)), (9, pf(qt[0], wq[0], xq, 3)),
            (11, pf(kt[1], wk[1], xk, 0)), (13, pf(kt[1], wk[1], xk, 1)),
        ])
        sweep(0, 2, fillers=[
            (3, nf(0, 1, 0)), (5, nf(0, 1, 1)),
            (7, pf(kt[1], wk[1], xk, 2)), (9, pf(kt[1], wk[1], xk, 3)),
            (11, pf(qt[1], wq[1], xq, 0)),
        ])
        sweep(0, 3, fillers=[
            (3, nf(0, 2, 0)), (5, nf(0, 2, 1)),
            (7, pf(qt[1], wq[1], xq, 1)), (9, pf(qt[1], wq[1], xq, 2)),
        ])
        sweep(1, 0, fillers=[
            (3, nf(0, 3, 0)), (5, nf(0, 3, 1)),
            (7, pf(qt[1], wq[1], xq, 3)),
        ])
        sweep(1, 1, fillers=[
            (3, nf(1, 0, 0)), (5, nf(1, 0, 1)),
            (7, of(0)), (9, of(1)), (11, of(2)), (13, of(3)),
        ])
        sweep(1, 2, fillers=[
            (3, nf(1, 1, 0)), (5, nf(1, 1, 1)),
            (7, of(4)), (9, of(5)), (11, of(6)), (13, of(7)),
        ])
        sweep(1, 3, fillers=[
            (3, nf(1, 2, 0)), (5, nf(1, 2, 1)),
            (7, of(8)), (9, of(9)), (11, of(10)), (13, of(11)),
        ])
        # tail
        for s in range(2):
            norm_part(1, 3, s)
        for t in range(12, 16):
            outproj(t)

    nc.compile()
    return nc


def _prep_core_inputs(c, x1, x2, v, Wq, Wk, Wv, Wo):
    bf = ml_dtypes.bfloat16
    b, g = c // 2, c % 2
    hs = slice(g * HPC, (g + 1) * HPC)
    wq = (Wq[hs] * (1.0 / np.sqrt(E))).astype(np.float32)   # fold 1/sqrt(E)
    wk, wv, wo = Wk[hs], Wv[hs], Wo[hs]

    def t_pack_pair(w):
        # [4,E,D] -> per pair p: concat(w[2p].T, w[2p+1].T, axis=1) [D,128]
        # -> k-subtile-major in the free dim: [2, 128, KT*128]
        out = np.empty((2, 128, KT * 128), bf)
        for p in range(2):
            m = np.concatenate([w[2 * p].T, w[2 * p + 1].T], axis=1)  # [D,128]
            out[p] = (m.reshape(KT, 128, 128).transpose(1, 0, 2)
                      .reshape(128, KT * 128).astype(bf))
        return out

    def x_chunks(x):
        # x[b].T [512, 2048] -> [chunk c, 128, KT*512]
        a = x.T.reshape(KT, 128, 4, 512).transpose(2, 1, 0, 3)
        return np.ascontiguousarray(a).astype(bf).reshape(4, 128, KT * 512)

    wvT = np.concatenate([wv[h].T for h in range(HPC)], axis=1)  # [D, 256]
    wvT = (wvT.reshape(KT, 128, HPC * E).transpose(1, 0, 2)
           .reshape(128, KT * HPC * E))
    # output weights packed in head pairs: [2, 2E=128, D]
    woP = np.stack([
        np.concatenate([wo[2 * p].T, wo[2 * p + 1].T], axis=0)
        for p in range(2)
    ])
    return {
        "xqT": x_chunks(x2[b]), "xkT": x_chunks(x1[b]), "vT": x_chunks(v[b]),
        "wqT": t_pack_pair(wq), "wkT": t_pack_pair(wk),
        "wvT": np.ascontiguousarray(wvT).astype(bf),
        "woP": woP.astype(bf),
    }


def kernel(**inputs):
    from concourse.bass_utils import run_bass_kernel_spmd

    x1 = np.asarray(inputs["x1"], np.float32)
    x2 = np.asarray(inputs["x2"], np.float32)
    v = np.asarray(inputs["v"], np.float32)
    Wq = np.asarray(inputs["Wq"], np.float32)
    Wk = np.asarray(inputs["Wk"], np.float32)
    Wv = np.asarray(inputs["Wv"], np.float32)
    Wo = np.asarray(inputs["Wo"], np.float32)

    if "nc" not in _CACHE:
        _CACHE["nc"] = _build()
    nc = _CACHE["nc"]

    in_maps = [
        _prep_core_inputs(c, x1, x2, v, Wq, Wk, Wv, Wo)
        for c in range(N_CORES)
    ]
    res = run_bass_kernel_spmd(nc, in_maps, list(range(N_CORES)))
    out = np.empty((B, N, D), np.float32)
    for b in range(B):
        out[b] = (
            res.results[2 * b]["outp"].reshape(N, D)
            + res.results[2 * b + 1]["outp"].reshape(N, D)
        )
    return out


# revision 21
# speedup vs baseline: 1.2442x; 1.2442x over previous
"""Trainium2 Bass kernel for nn_Attention_91293824844283.

Multi-head attention (identity rep): per-head 1x1-conv Q/K/V projections,
softmax(Q K^T / sqrt(E)) V, per-head output projection summed over heads.

Shapes: B=4, N=2048, D=512, H=8, E=64.

Sharding over 8 cores: core c -> (batch b = c//2, head-group g = c%2 of 4
heads). Each core computes the partial output sum over its 4 heads for its
batch; host adds the two partials per batch.

Device-side design (per core), v2:
  - Layouts as before: x^T inputs (bf16), packed transposed weights,
    1/sqrt(E) folded into Wq. V augmented with a ones column (slot 66) so
    the PV matmul also emits softmax denominators (M=65).
  - Scalar-engine exp is the pacer (128 ACTIVATEs x ~1.1us). All other
    work is emitted INSIDE the exp-paced sweeps so the in-order engine
    queues never stall ACT: quarter q's normalization + output projection
    run as PE/DVE fillers interleaved into quarter q+1's sweep.
  - Transpose-free normalization: rep stays unnormalized; r = 1/sums row
    (DVE reciprocal on [1,512]), PE outer-product broadcasts r to [64,512],
    one DVE multiply writes pre-normalized bf16 rep^T into packed pair
    tiles (head s at partitions s*64).
  - Output projection per 128-query tile: two concurrent K=64 row-tiled
    matmul chains (heads 0/2 -> bank A via rows 0:64, heads 1/3 -> bank B
    via rows 64:128), ost = A + B on DVE, DMA out.
  - Ramp: chunked K/Q/V projections interleaved with the first sweep's
    tiles so the first exp fires ~5us in.
"""

import numpy as np
import ml_dtypes
from contextlib import ExitStack

B, N, D, H, E = 4, 2048, 512, 8, 64
HPC = 4            # heads per core
N_CORES = 8
NKT = N // 128     # 16 nk tiles
VSLOT = 66         # V slot: 64 V cols + 1 ones col + 1 pad
KT = D // 128      # 4 contraction tiles for projections
QW = 512           # nq quarter width

_CACHE = {}


def _build():
    import concourse.tile as tile
    from concourse import bacc, mybir

    bf16 = mybir.dt.bfloat16
    f32 = mybir.dt.float32
    Exp = mybir.ActivationFunctionType.Exp

    nc = bacc.Bacc(
        "TRN2", target_bir_lowering=False, debug=False, num_devices=N_CORES
    )
    # chunk-major x layouts: [chunk c, 128, KT*512] so each chunk is ONE
    # contiguous 512KB DMA (serial dma_starts on the issuing engine block
    # for ~transfer time — fewer, bigger transfers keep the queue short).
    xqT = nc.dram_tensor("xqT", [4, 128, KT * 512], bf16, kind="ExternalInput").ap()
    xkT = nc.dram_tensor("xkT", [4, 128, KT * 512], bf16, kind="ExternalInput").ap()
    vT = nc.dram_tensor("vT", [4, 128, KT * 512], bf16, kind="ExternalInput").ap()
    wqT = nc.dram_tensor("wqT", [2, 128, KT * 128], bf16, kind="ExternalInput").ap()
    wkT = nc.dram_tensor("wkT", [2, 128, KT * 128], bf16, kind="ExternalInput").ap()
    wvT = nc.dram_tensor("wvT", [128, KT * HPC * E], bf16, kind="ExternalInput").ap()
    woP = nc.dram_tensor("woP", [2, 128, D], bf16, kind="ExternalInput").ap()
    outp = nc.dram_tensor("outp", [NKT, 128, D], f32, kind="ExternalOutput").ap()

    with tile.TileContext(nc) as tc, ExitStack() as ctx:
        cp = ctx.enter_context(tc.tile_pool(name="const", bufs=1))

        # --- persistent SBUF tiles (chunk-major x: tile c holds all KT
        # k-subtiles of 512 columns each) ---
        xq = [cp.tile([128, KT * 512], bf16, tag=f"xq{c}", name=f"xq{c}")
              for c in range(4)]
        xk = [cp.tile([128, KT * 512], bf16, tag=f"xk{c}", name=f"xk{c}")
              for c in range(4)]
        xv = [cp.tile([128, KT * 512], bf16, tag=f"xv{c}", name=f"xv{c}")
              for c in range(4)]
        wq = [cp.tile([128, KT * 128], bf16, tag=f"wq{p}", name=f"wq{p}")
              for p in range(2)]
        wk = [cp.tile([128, KT * 128], bf16, tag=f"wk{p}", name=f"wk{p}")
              for p in range(2)]
        wv = cp.tile([128, KT * HPC * E], bf16, tag="wv", name="wv")
        wo = [cp.tile([128, D], bf16, tag=f"wo{p}", name=f"wo{p}") for p in range(2)]
        qt = [cp.tile([128, N], bf16, tag=f"qt{p}", name=f"qt{p}") for p in range(2)]
        kt = [cp.tile([128, N], bf16, tag=f"kt{p}", name=f"kt{p}") for p in range(2)]
        vaug = [cp.tile([128, HPC * VSLOT], bf16, tag=f"va{t}", name=f"va{t}")
                for t in range(NKT)]
        # pre-normalized rep^T, packed pairs: head 2p+s at partitions s*64.
        # s=0 is written directly by DVE; s=1 lands in rsh (partitions
        # 0:64) and is shifted up via SBUF->SBUF DMA (DVE lanes are
        # partition-aligned; DMA can cross partitions).
        repP = [cp.tile([128, N], bf16, tag=f"rp{p}", name=f"repP{p}")
                for p in range(2)]
        rsh = [[cp.tile([E, QW], bf16, tag=f"rsh{par}{s}", name=f"rsh{par}{s}")
                for s in range(2)] for par in range(2)]
        # drain staging (double buffered by quarter parity)
        rts = [[cp.tile([65, QW], f32, tag=f"rts{par}{s}", name=f"rts{par}{s}")
                for s in range(2)] for par in range(2)]
        rcb = [[cp.tile([1, QW], bf16, tag=f"rcb{par}{s}", name=f"rcb{par}{s}")
                for s in range(2)] for par in range(2)]
        # [1,512] DVE reciprocal is pathological (one lane, iterative op,
        # ~3.3us). Round-trip the sums row through a [128,4] layout via
        # SBUF->SBUF DMAs (scatter p-major, gather back the same way — the
        # intermediate layout cancels) so recip runs on 128 lanes (~90ns).
        dT = [[cp.tile([128, 4], f32, tag=f"dT{par}{s}", name=f"dT{par}{s}")
               for s in range(2)] for par in range(2)]
        rT = [[cp.tile([128, 4], f32, tag=f"rT{par}{s}", name=f"rT{par}{s}")
               for s in range(2)] for par in range(2)]
        rTb = [[cp.tile([128, 4], bf16, tag=f"rTb{par}{s}", name=f"rTb{par}{s}")
                for s in range(2)] for par in range(2)]
        ones64 = cp.tile([1, 64], bf16, tag="ones64")
        dmy = cp.tile([128, 1], f32, tag="dmy")
        dmyo = cp.tile([128, 1], bf16, tag="dmyo")

        # --- exp table preload: first ACTIVATE triggers ACT_TABLE_LOAD
        # (~2.7us); fire it immediately so it overlaps the input DMA.
        nc.gpsimd.memset(dmy[:], 0.0)
        nc.scalar.activation(dmyo[:], dmy[:], Exp)
        nc.gpsimd.memset(ones64[:], 1.0)
        # ones columns for vaug on the (otherwise idle) vector engine so
        # gpsimd reaches the warmup memset quickly
        for t in range(NKT):
            nc.vector.memset(vaug[t][:], 1.0)

        # --- input DMAs, ordered to unblock the pipelined ramp:
        # pair-0 K weights + xk c0 first, then Q/V weights + c0, then the
        # remaining K/V chunks (vproj tiles follow the sweep), Q c1-3,
        # pair-1 weights, output-proj weights.
        # ALL input DMAs go on the sync queue: a dma_start occupies the
        # issuing engine for roughly the transfer time, so putting any on
        # the scalar queue would push the first exp out by ~20us.
        nc.sync.dma_start(wk[0][:], wkT[0])
        nc.sync.dma_start(xk[0][:], xkT[0])
        nc.sync.dma_start(wq[0][:], wqT[0])
        nc.sync.dma_start(xq[0][:], xqT[0])
        nc.sync.dma_start(wv[:], wvT[:])
        nc.sync.dma_start(xv[0][:], vT[0])
        for c in range(1, 4):
            nc.sync.dma_start(xk[c][:], xkT[c])
            nc.sync.dma_start(xv[c][:], vT[c])
        for c in range(1, 4):
            nc.sync.dma_start(xq[c][:], xqT[c])
        nc.sync.dma_start(wk[1][:], wkT[1])
        nc.sync.dma_start(wq[1][:], wqT[1])
        for p in range(2):
            nc.sync.dma_start(wo[p][:], woP[p])

        # --- small PE warmup burst during the initial DMA window.
        warm_sb = cp.tile([128, 512], bf16, tag="warm_sb")
        nc.gpsimd.memset(warm_sb[:], 0.0)
        with tc.tile_pool(name="warmps", bufs=1, space="PSUM") as wps:
            wpt = wps.tile([128, 512], f32, tag="w", name="warm_ps")
            for i in range(12):
                nc.tensor.matmul(wpt[:], warm_sb[:, 0:128], warm_sb[:],
                                 start=True, stop=True)

        # --- pools. PSUM: s pair tile 2 banks x bufs=2 + rep 2x1 bank +
        # fill 2x1 = 8 banks.
        sp = ctx.enter_context(tc.tile_pool(name="spsum", bufs=2, space="PSUM"))
        rp = ctx.enter_context(tc.tile_pool(name="rpsum", bufs=1, space="PSUM"))
        fpp = ctx.enter_context(tc.tile_pool(name="fill", bufs=2, space="PSUM"))
        ptp = ctx.enter_context(tc.tile_pool(name="ptile", bufs=4))
        ostp = ctx.enter_context(tc.tile_pool(name="ostp", bufs=3))

        def proj_chunk(dst, w, x, c):
            ps = fpp.tile([128, 512], f32, tag="f", name="proj_ps")
            for k in range(KT):
                nc.tensor.matmul(
                    ps[:], w[:, k * 128:(k + 1) * 128],
                    x[c][:, k * 512:(k + 1) * 512],
                    start=(k == 0), stop=(k == KT - 1),
                )
            nc.vector.tensor_copy(dst[:, c * 512:(c + 1) * 512], ps[:])

        def vproj(t):
            ps = fpp.tile([128, HPC * E], f32, tag="f", name="vproj_ps")
            c, off = t // 4, (t % 4) * 128
            for k in range(KT):
                nc.tensor.matmul(
                    ps[:], xv[c][:, k * 512 + off:k * 512 + off + 128],
                    wv[:, k * HPC * E:(k + 1) * HPC * E],
                    start=(k == 0), stop=(k == KT - 1),
                )
            for h in range(HPC):
                nc.vector.tensor_copy(
                    vaug[t][:, h * VSLOT:h * VSLOT + E],
                    ps[:, h * E:(h + 1) * E],
                )

        def norm_part(p, q, s):
            # quarter (p,q) head s: rts holds unnormalized rep^T [65, 512]
            # (row 64 = softmax denominators). Write pre-normalized bf16
            # rep^T into repP[p] partitions s*64:(s+1)*64.
            par = (4 * p + q) % 2
            qsl = slice(q * QW, (q + 1) * QW)
            nc.vector.reciprocal(rT[par][s][:], dT[par][s][:])
            nc.vector.tensor_copy(rTb[par][s][:], rT[par][s][:])
            nc.sync.dma_start(rcb[par][s][:], rTb[par][s][:])
            bc = fpp.tile([64, QW], f32, tag="f", name="bcast")
            nc.tensor.matmul(bc[:], ones64[:], rcb[par][s][:],
                             start=True, stop=True)
            if s == 0:
                nc.vector.tensor_mul(repP[p][0:E, qsl],
                                     rts[par][s][0:E, :], bc[:])
            else:
                nc.vector.tensor_mul(rsh[par][1][:],
                                     rts[par][s][0:E, :], bc[:])
                nc.sync.dma_start(repP[p][E:128, qsl], rsh[par][1][:])

        def outproj(t):
            tsl = slice(t * 128, (t + 1) * 128)
            pa = fpp.tile([128, D], f32, tag="f", name="opA")
            pb = fpp.tile([128, D], f32, tag="f", name="opB")
            for p in range(2):
                nc.tensor.matmul(pa[:], repP[p][0:E, tsl], wo[p][0:E, :],
                                 start=(p == 0), stop=(p == 1))
                nc.tensor.matmul(pb[:], repP[p][E:128, tsl], wo[p][E:128, :],
                                 start=(p == 0), stop=(p == 1))
            osa = ostp.tile([128, D], f32, tag="osa", name="osa")
            nc.vector.tensor_copy(osa[:], pa[:])
            ost = ostp.tile([128, D], f32, tag="ost", name="ost")
            nc.vector.tensor_add(ost[:], osa[:], pb[:])
            nc.sync.dma_start(outp[t], ost[:])

        def sweep(p, q, pre=None, fillers=()):
            """One attention quarter: 16 x (S pair, exp, PV pair).

            pre: dict t -> list of callables emitted before tile t's S.
            fillers: list of (t, fn): fn is emitted after tile t's PV (it
            executes in engine gaps while ACT paces the sweep). Slots must
            be late enough that any DMA the fn depends on has landed —
            a premature emission stalls the whole in-order PE queue.
            """
            fq = {}
            for slot, fn in fillers:
                fq.setdefault(slot, []).append(fn)
            qoff = q * QW
            rep = [rp.tile([65, QW], f32, tag=f"rep{s}", name=f"rep{s}")
                   for s in range(2)]
            def pv(t, pt):
                tsl = slice(t * 128, (t + 1) * 128)
                for s in range(2):
                    h = 2 * p + s
                    vsl = slice(h * VSLOT, h * VSLOT + 65)
                    nc.tensor.matmul(
                        rep[s][:],
                        vaug[t][:, vsl], pt[:, s * QW:(s + 1) * QW],
                        start=(t == 0), stop=(t == NKT - 1),
                    )

            # PV runs one tile behind S/exp so the next sweep's first S
            # isn't queued behind two PVs at the boundary (which would
            # stall the exp pacer ~1.3us per quarter).
            prev_pt = None
            for t in range(NKT):
                if pre:
                    for fn in pre.get(t, ()):
                        fn()
                tsl = slice(t * 128, (t + 1) * 128)
                spair = sp.tile([128, 2 * QW], f32, tag="s", name="spair")
                for s in range(2):
                    esl = slice(s * 64, (s + 1) * 64)
                    nc.tensor.matmul(
                        spair[:, s * QW:(s + 1) * QW],
                        kt[p][esl, tsl], qt[p][esl, qoff:qoff + QW],
                        start=True, stop=True,
                    )
                pt = ptp.tile([128, 2 * QW], bf16, tag="p", name="pt")
                nc.scalar.activation(pt[:], spair[:], Exp)
                if prev_pt is not None:
                    pv(t - 1, prev_pt)
                prev_pt = pt
                for fn in fq.get(t, ()):
                    fn()
            pv(NKT - 1, prev_pt)
            # drain rep -> rts staging so the next quarter can reuse the
            # rep PSUM banks; also kick off the sums-row scatter so the
            # reciprocal chain is ready early next sweep. Consumers run as
            # fillers next sweep.
            par = (4 * p + q) % 2
            for s in range(2):
                nc.vector.tensor_copy(rts[par][s][:], rep[s][:])
                nc.sync.dma_start(dT[par][s][:], rts[par][s][64:65, :])

        # --- ramp: minimal work to start sweep (0,0) ---
        proj_chunk(kt[0], wk[0], xk, 0)
        proj_chunk(qt[0], wq[0], xq, 0)
        for t in range(4):
            vproj(t)

        # --- emission schedule ---
        # sweep (0,0): in-sweep K-chunk projections + vproj per tile
        pre00 = {}
        for c in range(1, 4):
            pre00.setdefault(4 * c, []).append(
                (lambda cc: lambda: proj_chunk(kt[0], wk[0], xk, cc))(c))
        for t in range(4, NKT):
            pre00.setdefault(t, []).append((lambda tt: lambda: vproj(tt))(t))
        # qproj c1's input DMA lands ~15us in — slot it late in the sweep
        # so the in-order PE queue never blocks on it.
        sweep(0, 0, pre=pre00,
              fillers=[(13, lambda: proj_chunk(qt[0], wq[0], xq, 1))])

        def nf(p, q, s):
            return lambda: norm_part(p, q, s)

        def pf(dst, w, x, c):
            return lambda: proj_chunk(dst, w, x, c)

        def of(t):
            return lambda: outproj(t)

        sweep(0, 1, fillers=[
            (3, nf(0, 0, 0)), (5, nf(0, 0, 1)),
            (7, pf(qt[0], wq[0], xq, 2)), (9, pf(qt[0], wq[0], xq, 3)),
            (11, pf(kt[1], wk[1], xk, 0)), (13, pf(kt[1], wk[1], xk, 1)),
        ])
        sweep(0, 2, fillers=[
            (3, nf(0, 1, 0)), (5, nf(0, 1, 1)),
            (7, pf(kt[1], wk[1], xk, 2)), (9, pf(kt[1], wk[1], xk, 3)),
            (11, pf(qt[1], wq[1], xq, 0)),
        ])
        sweep(0, 3, fillers=[
            (3, nf(0, 2, 0)), (5, nf(0, 2, 1)),
            (7, pf(qt[1], wq[1], xq, 1)), (9, pf(qt[1], wq[1], xq, 2)),
        ])
        sweep(1, 0, fillers=[
            (3, nf(0, 3, 0)), (5, nf(0, 3, 1)),
            (7, pf(qt[1], wq[1], xq, 3)),
        ])
        sweep(1, 1, fillers=[
            (3, nf(1, 0, 0)), (5, nf(1, 0, 1)),
            (7, of(0)), (9, of(1)), (11, of(2)), (13, of(3)),
        ])
        sweep(1, 2, fillers=[
            (3, nf(1, 1, 0)), (5, nf(1, 1, 1)),
            (7, of(4)), (9, of(5)), (11, of(6)), (13, of(7)),
        ])
        sweep(1, 3, fillers=[
            (3, nf(1, 2, 0)), (5, nf(1, 2, 1)),
            (7, of(8)), (9, of(9)), (11, of(10)), (13, of(11)),
        ])
        # tail
        for s in range(2):
            norm_part(1, 3, s)
        for t in range(12, 16):
            outproj(t)

    nc.compile()
    return nc


def _prep_core_inputs(c, x1, x2, v, Wq, Wk, Wv, Wo):
    bf = ml_dtypes.bfloat16
    b, g = c // 2, c % 2
    hs = slice(g * HPC, (g + 1) * HPC)
    wq = (Wq[hs] * (1.0 / np.sqrt(E))).astype(np.float32)   # fold 1/sqrt(E)
    wk, wv, wo = Wk[hs], Wv[hs], Wo[hs]

    def t_pack_pair(w):
        # [4,E,D] -> per pair p: concat(w[2p].T, w[2p+1].T, axis=1) [D,128]
        # -> k-subtile-major in the free dim: [2, 128, KT*128]
        out = np.empty((2, 128, KT * 128), bf)
        for p in range(2):
            m = np.concatenate([w[2 * p].T, w[2 * p + 1].T], axis=1)  # [D,128]
            out[p] = (m.reshape(KT, 128, 128).transpose(1, 0, 2)
                      .reshape(128, KT * 128).astype(bf))
        return out

    def x_chunks(x):
        # x[b].T [512, 2048] -> [chunk c, 128, KT*512]
        a = x.T.reshape(KT, 128, 4, 512).transpose(2, 1, 0, 3)
        return np.ascontiguousarray(a).astype(bf).reshape(4, 128, KT * 512)

    wvT = np.concatenate([wv[h].T for h in range(HPC)], axis=1)  # [D, 256]
    wvT = (wvT.reshape(KT, 128, HPC * E).transpose(1, 0, 2)
           .reshape(128, KT * HPC * E))
    # output weights packed in head pairs: [2, 2E=128, D]
    woP = np.stack([
        np.concatenate([wo[2 * p].T, wo[2 * p + 1].T], axis=0)
        for p in range(2)
    ])
    return {
        "xqT": x_chunks(x2[b]), "xkT": x_chunks(x1[b]), "vT": x_chunks(v[b]),
        "wqT": t_pack_pair(wq), "wkT": t_pack_pair(wk),
        "wvT": np.ascontiguousarray(wvT).astype(bf),
        "woP": woP.astype(bf),
    }


def kernel(**inputs):
    from concourse.bass_utils import run_bass_kernel_spmd

    x1 = np.asarray(inputs["x1"], np.float32)
    x2 = np.asarray(inputs["x2"], np.float32)
    v = np.asarray(inputs["v"], np.float32)
    Wq = np.asarray(inputs["Wq"], np.float32)
    Wk = np.asarray(inputs["Wk"], np.float32)
    Wv = np.asarray(inputs["Wv"], np.float32)
    Wo = np.asarray(inputs["Wo"], np.float32)

    if "nc" not in _CACHE:
        _CACHE["nc"] = _build()
    nc = _CACHE["nc"]

    in_maps = [
        _prep_core_inputs(c, x1, x2, v, Wq, Wk, Wv, Wo)
        for c in range(N_CORES)
    ]
    res = run_bass_kernel_spmd(nc, in_maps, list(range(N_CORES)))
    out = np.empty((B, N, D), np.float32)
    for b in range(B):
        out[b] = (
            res.results[2 * b]["outp"].reshape(N, D)
            + res.results[2 * b + 1]["outp"].reshape(N, D)
        )
    return out
